# revision 24
# baseline (speedup 1.0000x reference)
"""DenseFastGAT forward on 8 Trainium2 NeuronCores (Bass/Tile).

Math (per batch b):
  z  = x @ W.T + bW                                  [N, O]
  ai = z @ wai.T + bai ; aj = z @ waj.T + baj        [N]
  e  = leakyrelu(ai_i + aj_j, 0.2)
  att = softmax_row(where(adj>0, e, -9e15) ++ sink(-1e9))[:, :N]
  out = att @ z

Kernel strategy (v5):
  - ai/aj fold to x @ (W.T @ wai.T) + const on host (f64, tiny).
  - Sharding: 8 cores = 2 batches x 4 row-slabs of NI=1024 rows each.
  - Re-association: out = (att @ [x|1]) @ W.T + bW. Main loop
    accumulates Y = p.T @ [x|1] (col 256 = softmax denominator); tail
    projects yhat = Y/d through W.T via PE transpose + 16 matmuls; bW
    added on host at unshard (softmax rows sum to 1, exact).
  - p field (rows scale-invariant; row i scaled by exp(-0.2*ai_i)):
      p'[j,i] = adj * max(exp(0.8*ai_i + aj_j), exp(0.2*aj_j))
    Measured DVE rates/tile: ts_mul 0.48us (4x), tensor_tensor 0.57
    (2x), scalar_tensor_tensor 1.28 (1x), ts_max 1.09 (1x); ACT exp
    1.1; Pool tt 2.27 (bf16 only - fp8 drops every engine to 1x).
    Per-quad split (pace ~4.4us, PE matmuls 3.5us):
      k0: DVE ts_mul(g_bc) -> DVE stt (max f2, mult adj)
      k1,k2: ACT Exp(0.8*ai+aj) -> DVE stt
      k3: ACT Relu(0.8*(ai+aj)) -> ACT Exp(r + 0.2*aj)  [max folded:
          exp(0.2aj + relu(0.8(ai+aj))) == max branch, exact]
          -> Pool tensor_tensor mask
  - Startup: Sync queue carries g_bc/f1c/adj-jt0/x-q0 first (DMA
    setups serialize ~0.7us); ACT-route consts ride the gpsimd queue.
  - No max-subtraction softmax: all fields positive, denominators
    >= 20*exp(-3); bf16 covers exp(27).
"""

import numpy as np
import ml_dtypes

B = 2
N = 4096
IN_F = 256
O = 256
NCORES = 8
SLABS_PER_B = 4
NI = N // SLABS_PER_B        # 1024 rows per core
JT = N // 128                # 32 j-tiles
NQ = JT // 4                 # 8 quads of j-tiles
IC = NI // 128               # 8 output chunks per core
KA = IN_F + 1                # 257 (x augmented with ones column)
ALPHA = 0.2

_CACHE = {}


def _build():
    import concourse.bacc as bacc
    import concourse.mybir as mybir
    import concourse.tile as tile

    dt = mybir.dt
    AF = mybir.ActivationFunctionType
    ALU = mybir.AluOpType

    nc = bacc.Bacc("TRN2", target_bir_lowering=False, debug=False,
                   num_devices=NCORES)

    adjsT = nc.dram_tensor("adjsT", [N, NI], dt.bfloat16, kind="ExternalInput")
    x_aug = nc.dram_tensor("x_aug", [128, JT, KA], dt.bfloat16,
                           kind="ExternalInput")
    wT_d = nc.dram_tensor("wT", [128, 2, O], dt.bfloat16, kind="ExternalInput")
    g_d = nc.dram_tensor("g_bc", [128, NI], dt.bfloat16, kind="ExternalInput")
    ai_d = nc.dram_tensor("ai_bc", [128, NI], dt.float32,
                          kind="ExternalInput")
    f1c_d = nc.dram_tensor("f1c", [128, JT], dt.float32, kind="ExternalInput")
    f2c_d = nc.dram_tensor("f2c", [128, JT], dt.float32, kind="ExternalInput")
    ajc_d = nc.dram_tensor("ajc", [128, JT], dt.float32, kind="ExternalInput")
    aj08_d = nc.dram_tensor("aj08", [128, JT], dt.float32,
                            kind="ExternalInput")
    aj02_d = nc.dram_tensor("aj02", [128, JT], dt.float32,
                            kind="ExternalInput")
    ident_d = nc.dram_tensor("ident", [128, 128], dt.bfloat16,
                             kind="ExternalInput")
    out = nc.dram_tensor("out", [NI, O], dt.float32, kind="ExternalOutput")

    adjq_view = adjsT.ap().rearrange("(q k p) i -> q p k i", k=4, p=128)
    adjj_view = adjsT.ap().rearrange("(t p) i -> t p i", p=128)

    with tile.TileContext(nc) as tc:
        with tc.tile_pool(name="consts", bufs=1) as consts, \
             tc.tile_pool(name="adjp", bufs=5) as adjp, \
             tc.tile_pool(name="tvp", bufs=4) as tvp, \
             tc.tile_pool(name="rkp", bufs=2) as rkp, \
             tc.tile_pool(name="pp", bufs=4) as pp, \
             tc.tile_pool(name="ysbp", bufs=1) as ysbp, \
             tc.tile_pool(name="ytp", bufs=4) as ytp, \
             tc.tile_pool(name="outp", bufs=8) as outp, \
             tc.tile_pool(name="smallp", bufs=2) as smallp:

            g_bc = consts.tile([128, NI], dt.bfloat16, tag="g_bc")
            ai_bc = consts.tile([128, NI], dt.float32, tag="ai_bc")
            f1c = consts.tile([128, JT], dt.float32, tag="f1c")
            f2c = consts.tile([128, JT], dt.float32, tag="f2c")
            ajc = consts.tile([128, JT], dt.float32, tag="ajc")
            aj08 = consts.tile([128, JT], dt.float32, tag="aj08")
            aj02 = consts.tile([128, JT], dt.float32, tag="aj02")
            wT_sb = consts.tile([128, 2, O], dt.bfloat16, tag="wT")
            ident = consts.tile([128, 128], dt.bfloat16, tag="ident")
            x_sb = consts.tile([128, JT, KA], dt.bfloat16, tag="x_sb")

            # DVE-route consts first on the Sync queue
            nc.sync.dma_start(out=g_bc[:], in_=g_d[:])
            nc.sync.dma_start(out=f1c[:], in_=f1c_d[:])
            nc.sync.dma_start(out=f2c[:], in_=f2c_d[:])
            # ACT-route consts on the gpsimd queue (parallel setups).
            # ai_bc (512K, the fattest const) goes LAST so it does not
            # contend with g_bc/adj-jt0 in the first-transfer window.
            nc.gpsimd.dma_start(out=ajc[:], in_=ajc_d[:])
            nc.gpsimd.dma_start(out=aj08[:], in_=aj08_d[:])
            nc.gpsimd.dma_start(out=aj02[:], in_=aj02_d[:])
            nc.gpsimd.dma_start(out=ai_bc[:], in_=ai_d[:])

            # ---- main loop: Y[ic] += p'.T @ [x|1] over 32 j-tiles ----
            with tc.tile_pool(name="accp", bufs=1, space="PSUM") as accp:
                accs = [accp.tile([128, KA], dt.float32, tag=f"acc{ic}",
                                  name=f"acc{ic}")
                        for ic in range(IC)]
                r_t = smallp.tile([128, IC], dt.float32, tag="r_t")
                ysb = ysbp.tile([128, IC, O], dt.bfloat16, tag="ysb")

                for q in range(NQ):
                    adjt = adjp.tile([128, 4, NI], dt.bfloat16, name="adjt")
                    if q == 0:
                        nc.sync.dma_start(out=adjt[:, 0, :], in_=adjj_view[0])
                        nc.sync.dma_start(out=x_sb[:, 0:4, :],
                                          in_=x_aug[:, 0:4, :])
                        for k in range(1, 4):
                            nc.sync.dma_start(out=adjt[:, k, :],
                                              in_=adjj_view[k])
                        # tail-only consts; issued early, needed late
                        nc.sync.dma_start(out=wT_sb[:], in_=wT_d[:])
                        nc.sync.dma_start(out=ident[:], in_=ident_d[:])
                    else:
                        nc.sync.dma_start(out=adjt[:], in_=adjq_view[q])
                        nc.sync.dma_start(out=x_sb[:, 4 * q:4 * q + 4, :],
                                          in_=x_aug[:, 4 * q:4 * q + 4, :])

                    tv = tvp.tile([128, 4, NI], dt.bfloat16, name="tv")
                    p_t = pp.tile([128, 4, NI], dt.bfloat16, name="p_t")
                    # k0/k1: build + in-place ts_max, then ONE merged
                    # 2x-mode mask tt (2x ts_max + tt/2 beats a 1x stt)
                    j0 = q * 4
                    nc.vector.tensor_scalar_mul(tv[:, 0, :], g_bc[:],
                                                f1c[:, j0:j0 + 1])
                    if q == 0:
                        # quad 0: k0 lane masks alone so its matmuls fire on
                        # the early small transfers (g_bc/f1c/f2c/adj-jt0)
                        # without waiting for the ACT route's ai_bc (which
                        # is deliberately the last const transfer)
                        nc.vector.tensor_scalar_max(tv[:, 0, :], tv[:, 0, :],
                                                    f2c[:, j0:j0 + 1])
                        nc.vector.tensor_tensor(p_t[:, 0, :], tv[:, 0, :],
                                                adjt[:, 0, :], op=ALU.mult)
                        nc.scalar.activation(tv[:, 1, :], ai_bc[:], AF.Exp,
                                             bias=ajc[:, j0 + 1:j0 + 2],
                                             scale=0.8)
                        nc.vector.tensor_scalar_max(tv[:, 1, :], tv[:, 1, :],
                                                    f2c[:, j0 + 1:j0 + 2])
                        nc.vector.tensor_tensor(p_t[:, 1, :], tv[:, 1, :],
                                                adjt[:, 1, :], op=ALU.mult)
                    else:
                        nc.scalar.activation(tv[:, 1, :], ai_bc[:], AF.Exp,
                                             bias=ajc[:, j0 + 1:j0 + 2],
                                             scale=0.8)
                        nc.vector.tensor_scalar_max(tv[:, 0, :], tv[:, 0, :],
                                                    f2c[:, j0:j0 + 1])
                        nc.vector.tensor_scalar_max(tv[:, 1, :], tv[:, 1, :],
                                                    f2c[:, j0 + 1:j0 + 2])
                        nc.vector.tensor_tensor(p_t[:, 0:2, :], tv[:, 0:2, :],
                                                adjt[:, 0:2, :], op=ALU.mult)
                    # k2: ACT exp + fused DVE stt
                    js2 = slice(j0 + 2, j0 + 3)
                    nc.scalar.activation(tv[:, 2, :], ai_bc[:], AF.Exp,
                                         bias=ajc[:, js2], scale=0.8)
                    nc.vector.scalar_tensor_tensor(
                        p_t[:, 2, :], tv[:, 2, :], f2c[:, js2],
                        adjt[:, 2, :], op0=ALU.max, op1=ALU.mult)
                    # k3: max folded on ACT (exp(0.2aj + relu(0.8(ai+aj))),
                    # relu intermediate in f32) + Pool mask
                    js3 = slice(j0 + 3, j0 + 4)
                    rk = rkp.tile([128, NI], dt.float32, name="rk")
                    nc.scalar.activation(rk[:], ai_bc[:], AF.Relu,
                                         bias=aj08[:, js3], scale=0.8)
                    nc.scalar.activation(tv[:, 3, :], rk[:], AF.Exp,
                                         bias=aj02[:, js3])
                    nc.gpsimd.tensor_tensor(p_t[:, 3, :], tv[:, 3, :],
                                            adjt[:, 3, :], op=ALU.mult)

                    if q < NQ - 1:
                        for k in range(4):
                            jt = q * 4 + k
                            for ic in range(IC):
                                nc.tensor.matmul(
                                    accs[ic][:],
                                    p_t[:, k, ic * 128:(ic + 1) * 128],
                                    x_sb[:, jt, :],
                                    start=(jt == 0), stop=False)
                    else:
                        # last quad ic-major: each acc finishes early so its
                        # normalize-cast pipelines under remaining matmuls
                        for ic in range(IC):
                            for k in range(4):
                                jt = q * 4 + k
                                nc.tensor.matmul(
                                    accs[ic][:],
                                    p_t[:, k, ic * 128:(ic + 1) * 128],
                                    x_sb[:, jt, :],
                                    start=False, stop=(k == 3))
                            nc.vector.reciprocal(r_t[:, ic:ic + 1],
                                                 accs[ic][:, O:O + 1])
                            if ic % 2 == 0:
                                nc.scalar.activation(ysb[:, ic, :],
                                                     accs[ic][:, 0:O],
                                                     AF.Copy,
                                                     scale=r_t[:, ic:ic + 1])
                            else:
                                nc.vector.tensor_scalar_mul(
                                    ysb[:, ic, :], accs[ic][:, 0:O],
                                    r_t[:, ic:ic + 1])

            # ---- tail: out = yhat @ W.T (PE transpose + 16 matmuls);
            # bW added on host during unshard ----
            with tc.tile_pool(name="tps", bufs=4, space="PSUM") as tps, \
                 tc.tile_pool(name="g2p", bufs=4, space="PSUM") as g2p:
                for ic in range(IC):
                    yt = ytp.tile([128, 2, 128], dt.bfloat16, name="yt")
                    for fh in range(2):
                        tp = tps.tile([128, 128], dt.bfloat16, name="tp")
                        nc.tensor.transpose(
                            tp[:], ysb[:, ic, fh * 128:(fh + 1) * 128],
                            ident[:])
                        if fh == 0:
                            nc.vector.tensor_copy(yt[:, fh, :], tp[:])
                        else:
                            nc.scalar.copy(yt[:, fh, :], tp[:])
                    G = g2p.tile([128, O], dt.float32, name="G")
                    nc.tensor.matmul(G[:], yt[:, 0, :], wT_sb[:, 0, :],
                                     start=True, stop=False)
                    nc.tensor.matmul(G[:], yt[:, 1, :], wT_sb[:, 1, :],
                                     start=False, stop=True)
                    ot = outp.tile([128, O], dt.float32, name="ot")
                    if ic % 2 == 0:
                        nc.vector.tensor_copy(ot[:], G[:])
                    else:
                        nc.scalar.copy(ot[:], G[:])
                    deng = nc.sync if ic % 2 == 0 else nc.gpsimd
                    deng.dma_start(out=out[ic * 128:(ic + 1) * 128, :],
                                   in_=ot[:])

    nc.compile()
    return nc


def _get_nc():
    if "nc" not in _CACHE:
        _CACHE["nc"] = _build()
    return _CACHE["nc"]


def kernel(x, adjs, W, bW, wai, bai, waj, baj):
    from concourse import bass_utils

    bf16 = ml_dtypes.bfloat16
    x = np.asarray(x, np.float32)
    adjs = np.asarray(adjs, np.float32)
    W = np.asarray(W, np.float32)
    bW = np.asarray(bW, np.float32)
    wai = np.asarray(wai, np.float32)
    bai = np.asarray(bai, np.float32)
    waj = np.asarray(waj, np.float32)
    baj = np.asarray(baj, np.float32)

    # host-folded attention projections (f64 for accuracy)
    u_i = W.astype(np.float64).T @ wai.astype(np.float64).T        # [256,1]
    c_i = float(bW.astype(np.float64) @ wai[0].astype(np.float64)
                + bai.astype(np.float64)[0])
    u_j = W.astype(np.float64).T @ waj.astype(np.float64).T
    c_j = float(bW.astype(np.float64) @ waj[0].astype(np.float64)
                + baj.astype(np.float64)[0])
    ai = (x.astype(np.float64) @ u_i)[:, :, 0] + c_i               # [B,N] f64
    aj = (x.astype(np.float64) @ u_j)[:, :, 0] + c_j

    # per-batch shared inputs
    wT = np.ascontiguousarray(
        W.T.reshape(2, 128, O).transpose(1, 0, 2)).astype(bf16)
    ident = np.eye(128, dtype=np.float32).astype(bf16)

    x_aug_b, f1_b, f2_b, aj_b, aj08_b, aj02_b = [], [], [], [], [], []
    for b in range(B):
        xa = np.empty((128, JT, KA), bf16)
        xa[:, :, :IN_F] = x[b].reshape(JT, 128, IN_F).transpose(1, 0, 2)
        xa[:, :, IN_F] = np.float32(1.0)
        x_aug_b.append(xa)
        ajr = aj[b].reshape(JT, 128).T
        f1_b.append(np.ascontiguousarray(np.exp(ajr)).astype(np.float32))
        f2_b.append(np.ascontiguousarray(np.exp(ALPHA * ajr)).astype(np.float32))
        aj_b.append(np.ascontiguousarray(ajr).astype(np.float32))
        aj08_b.append(np.ascontiguousarray(0.8 * ajr).astype(np.float32))
        aj02_b.append(np.ascontiguousarray(0.2 * ajr).astype(np.float32))

    in_maps = []
    for c in range(NCORES):
        b, s = divmod(c, SLABS_PER_B)
        i0 = s * NI
        adjsT_slab = np.ascontiguousarray(adjs[b][i0:i0 + NI, :].T).astype(bf16)
        ai_slab = ai[b, i0:i0 + NI]
        g_bc = np.broadcast_to(
            np.exp(0.8 * ai_slab).astype(bf16).reshape(1, NI), (128, NI))
        ai_bc = np.broadcast_to(
            ai_slab.astype(np.float32).reshape(1, NI), (128, NI))
        in_maps.append({
            "adjsT": adjsT_slab,
            "x_aug": x_aug_b[b],
            "wT": wT,
            "g_bc": np.ascontiguousarray(g_bc),
            "ai_bc": np.ascontiguousarray(ai_bc),
            "f1c": f1_b[b],
            "f2c": f2_b[b],
            "ajc": aj_b[b],
            "aj08": aj08_b[b],
            "aj02": aj02_b[b],
            "ident": ident,
        })

    nc = _get_nc()
    res = bass_utils.run_bass_kernel_spmd(
        nc, in_maps, core_ids=list(range(NCORES)),
        **_CACHE.get("run_kwargs", {}))
    _CACHE["last_results"] = res

    out = np.empty((B, N, O), np.float32)
    for c in range(NCORES):
        b, s = divmod(c, SLABS_PER_B)
        out[b, s * NI:(s + 1) * NI, :] = res.results[c]["out"] + bW
    return out
